# revision 37
# baseline (speedup 1.0000x reference)
"""Trainium2 Bass kernel for a 2-layer GCN (gnn_message_passing).

Reference computation (all f32 inputs):
    h      = relu(adj @ (x @ W1) + b1)        adj: [N, N], x: [N, F]
    logits = adj @ (h @ W2) + b2
    out    = log_softmax(logits, axis=1)       out: [N, C]

Distribution: 1-D row partition over 8 NeuronCores. Core i owns rows
R0 = i*N/8 .. R0+N/8. Because adj is symmetric (by construction), the
column slice adj[:, rows_i] in natural row-major layout is exactly the
transposed operand adj_i^T the TensorEngine needs as its moving operand,
so no on-chip transpose of adj is ever required.

Per-core plan (single NEFF launch, two AllGathers):
  - adj[:, rows_i] streamed via SWDGE cast-DMAs (f32 -> bf16 in flight,
    ~HBM line rate): host pre-permutes rows so each partition's slice of
    a superchunk is one KK*4KB-contiguous descriptor. adj stays RESIDENT
    in SBUF (16MB bf16) so layer 2 re-uses it with zero extra HBM traffic.
  - x_i^T loaded the same way (host-transposed, cast in flight). The
    S-AllGather trigger is interleaved into the gpsimd queue after only
    4 adj descriptor-gens so it fires ~15us in (the SWDGE ring paces
    desc-gen at drain speed; queueing it last would delay it ~100us).
  - layer 1: hT[f, m] accumulated in PSUM over all 64 k-chunks as the
    adj chunks land (DMA-gated).
  - z_i = h_i @ W2 -> AllGather z (bf16, tiny), PE warm-keeper spans the
    collective gap so layer 2 starts at full clock.
  - layer 2: logitsT from resident adj + gathered z, 2x column-tiled on
    the PE (40-wide output uses col-groups 0-1 for m-block 0 and 2-3 for
    m-block 1 concurrently -> ~2x), +b2, PE-transpose to [m, c],
    log_softmax on-chip, single output DMA.

kernel(**inputs) takes FULL inputs and returns the FULL [N, C] output.
"""

import numpy as np

import concourse.bass as bass
import concourse.mybir as mybir
import concourse.tile as tile
from concourse import bacc
from concourse.bass_utils import run_bass_kernel_spmd
from concourse.masks import make_identity

NCORES = 8
N_FULL = 8192
NFEAT = 512
NHID = 128
NCLASS = 40
F32 = mybir.dt.float32
BF16 = mybir.dt.bfloat16

KK = 2            # adj k-chunks per HWDGE DMA (8KB contiguous per partition)
WARM_N = 160      # PE warm-keeper matmuls (N=128) spanning the z-AllGather
L2_COLTILE = True
SB = [0, 3, 6, 8]  # S-AllGather group boundaries in k%MC units ([3,3,2] chunks)


def _korder(K: int, MC: int):
    """Layer-1 k consumption order: S group q of every rank's chunks first.
    The host streams adj chunks in this same order, so the PE never waits
    on a late adj chunk once the matching S group has arrived."""
    order = []
    for q in range(len(SB) - 1):
        order += [k for k in range(K) if SB[q] <= k % MC < SB[q + 1]]
    return order


def build(n_total: int = N_FULL):
    """Build the SPMD Bass graph for one core (same program on all 8)."""
    M = n_total // NCORES          # rows owned by this core
    K = n_total // 128             # 128-row contraction chunks
    MC = M // 128                  # 128-row output chunks on this core
    MW = min(512, M)               # moving free-dim width for the big matmuls
    MH = M // MW                   # number of row groups of width MW
    DF = NFEAT // 128              # feature chunks (4)

    nc = bacc.Bacc(
        "TRN2", target_bir_lowering=False, debug=False,
        enable_asserts=True, num_devices=NCORES,
    )

    xt = nc.dram_tensor("xt", [NFEAT, M], F32, kind="ExternalInput")
    adjc = nc.dram_tensor("adjc", [n_total, M], F32, kind="ExternalInput")
    w1 = nc.dram_tensor("w1", [NFEAT, NHID], F32, kind="ExternalInput")
    b1 = nc.dram_tensor("b1", [NHID, 1], F32, kind="ExternalInput")
    w2 = nc.dram_tensor("w2", [NHID, NCLASS], F32, kind="ExternalInput")
    b2 = nc.dram_tensor("b2", [NCLASS, 1], F32, kind="ExternalInput")
    out_ext = nc.dram_tensor("out", [M, NCLASS], F32, kind="ExternalOutput")

    rg = [list(range(NCORES))]

    with tile.TileContext(nc) as tc:
        with (
            tc.tile_pool(name="resident", bufs=1) as res,
            tc.tile_pool(name="dram", bufs=1, space="DRAM") as dram,
        ):
            adjres = res.tile([128, K * M], BF16)          # adj_i^T, bf16, resident
            sres = res.tile([128, K, NHID], BF16)          # gathered S, k-chunk layout
            zres = res.tile([128, K, NCLASS], BF16)        # gathered z, k-chunk layout
            hT = res.tile([128, M], BF16)                  # layer-1 out, [f, m]
            xtb = res.tile([128, DF, M], BF16)             # x_i^T bf16
            sloc = res.tile([128, MC, NHID], BF16)         # local S rows
            zloc = res.tile([128, MC, NCLASS], BF16)
            w1st = res.tile([128, DF, NHID], F32)
            w1bf = res.tile([128, DF, NHID], BF16)
            w2st = res.tile([128, NCLASS], F32)
            w2bf = res.tile([128, NCLASS], BF16)
            b1sb = res.tile([128, 1], F32)
            b2sb = res.tile([NCLASS, 1], F32)
            b2hi = res.tile([128, 1], F32)                 # b2 dup at partitions 64..103
            ident = res.tile([128, 128], F32)
            ident2 = res.tile([128, NCLASS], F32)          # shifted identity (64..103)
            lTsb = res.tile([NCLASS, M], F32)              # logits^T m-block 0 (+b2)
            lThi = res.tile([128, MW], F32)                # logits^T m-block 1 @64..103
            osb = res.tile([128, MC, NCLASS], F32)         # final log-softmax out

            # collective bounce buffers (internal DRAM)
            s_in = dram.tile([M, NHID], BF16)
            NG = len(SB) - 1
            sq_out = [
                dram.tile([NCORES * (SB[q + 1] - SB[q]) * 128, NHID], BF16,
                          addr_space="Shared", name=f"sq_out{q}")
                for q in range(NG)
            ]
            z_in = dram.tile([M, NCLASS], BF16)
            z_out = dram.tile([n_total, NCLASS], BF16, addr_space="Shared")

            # ---- gpsimd SWDGE queue carries ONLY: x^T cast-DMA, the s_in
            # bounce write, the collective triggers, and the sres load —
            # so the S-AllGather trigger fires as soon as S is ready
            # (~15us). The adj stream lives on the two HWDGE queues. ----
            nc.gpsimd.dma_start(
                out=xtb[:, :, :],
                in_=xt.ap().rearrange("(a p) m -> p a m", p=128),
            )

            # ---- constants on the sync HWDGE queue (first, they're tiny) ----
            nc.sync.dma_start(
                out=w1st[:, :, :],
                in_=w1.ap().rearrange("(a p) f -> p a f", p=128),
            )
            nc.sync.dma_start(out=b1sb[:, :], in_=b1.ap())
            nc.sync.dma_start(out=b2sb[:, :], in_=b2.ap())
            nc.sync.dma_start(out=b2hi[64:64 + NCLASS, :], in_=b2.ap())
            nc.sync.dma_start(out=w2st[:, :], in_=w2.ap())
            nc.vector.tensor_copy(w1bf[:, :, :], w1st[:, :, :])
            nc.vector.tensor_copy(w2bf[:, :], w2st[:, :])
            make_identity(nc, ident[:, :])
            make_identity(nc, ident2[64:64 + NCLASS, :])

            # ---- adj stream: f32 chunks on sync+scalar HWDGE, vector casts
            # into the bf16 resident tile ----
            astage_ctx = tc.tile_pool(name="astage", bufs=3)
            astage = astage_ctx.__enter__()
            for kk in range(K // KK):
                ast = astage.tile([128, KK, M], F32, tag="ast")
                dma_eng = nc.sync if kk % 2 == 0 else nc.scalar
                dma_eng.dma_start(
                    out=ast[:, :, :],
                    in_=adjc[kk * KK * 128:(kk + 1) * KK * 128, :].rearrange(
                        "(p a) m -> p a m", p=128
                    ),
                )
                nc.vector.tensor_copy(
                    adjres[:, kk * KK * M:(kk + 1) * KK * M],
                    ast.rearrange("p a m -> p (a m)"),
                )

            # ---- S phase: S_i = x_i @ W1 (stationary x^T chunks) ----
            with tc.tile_pool(name="spsum", bufs=2, space="PSUM") as spsum:
                for nci in range(MC):
                    ps = spsum.tile([128, NHID], F32, tag="ps")
                    for d in range(DF):
                        nc.tensor.matmul(
                            ps[:, :],
                            xtb[:, d, nci * 128:(nci + 1) * 128],
                            w1bf[:, d, :],
                            start=(d == 0), stop=(d == DF - 1),
                        )
                    nc.vector.tensor_copy(sloc[:, nci, :], ps[:, :])
            nc.gpsimd.dma_start(
                out=s_in.rearrange("(a p) f -> p a f", p=128),
                in_=sloc[:, :, :],
            )
            # S-AllGather split into groups (SB boundaries): layer 1 starts
            # on group 0's k-chunks while the rest are in flight; the group
            # sizes balance per-op latency vs serialization.
            for q in range(NG):
                nc.gpsimd.collective_compute(
                    "AllGather", mybir.AluOpType.bypass, replica_groups=rg,
                    ins=[s_in[SB[q] * 128:SB[q + 1] * 128, :]],
                    outs=[sq_out[q][:, :]],
                )
            for q in range(NG):
                JQ = SB[q + 1] - SB[q]
                for r in range(NCORES):
                    nc.gpsimd.dma_start(
                        out=sres[:, r * MC + SB[q]:r * MC + SB[q + 1], :],
                        in_=sq_out[q][r * JQ * 128:(r + 1) * JQ * 128, :]
                        .rearrange("(j p) f -> p j f", p=128),
                    )
            korder = _korder(K, MC)

            # ---- layer 1: hT += S_k^T @ adjT_k (DMA-gated on adj chunks) ----
            # adjres slot s holds adj chunk korder[s] (host streams chunks in
            # korder), so layer 1 walks slots in order while indexing sres by
            # the global chunk id. m-block 0 runs to completion first so its
            # z rows ship in an early z-AllGather while m-block 1 computes.
            HM = M // 2
            JH = HM // 128          # k-chunks per rank-half (4)
            HC = MC // 2
            with (
                tc.tile_pool(name="hpsum", bufs=1, space="PSUM") as hp,
                tc.tile_pool(name="zpsum", bufs=2, space="PSUM") as zp,
            ):
                ph = [hp.tile([128, MW], F32, name=f"ph{m}") for m in range(MH)]

                def l1_mm(mh, s):
                    nc.tensor.matmul(
                        ph[mh][:, :],
                        sres[:, korder[s], :],
                        adjres[:, s * M + mh * MW:s * M + (mh + 1) * MW],
                        start=(s == 0), stop=(s == K - 1),
                    )

                def z_half(mh):
                    nc.scalar.activation(
                        hT[:, mh * MW:(mh + 1) * MW], ph[mh][:, :],
                        mybir.ActivationFunctionType.Relu,
                        bias=b1sb[:, 0:1], scale=1.0,
                    )
                    for mc in range(mh * HC, (mh + 1) * HC):
                        pz = zp.tile([128, NCLASS], F32, tag="pz")
                        nc.tensor.matmul(
                            pz[:, :],
                            hT[:, mc * 128:(mc + 1) * 128],
                            w2bf[:, :],
                            start=True, stop=True,
                        )
                        nc.vector.tensor_copy(zloc[:, mc, :], pz[:, :])
                    nc.gpsimd.dma_start(
                        out=z_in[mh * HM:(mh + 1) * HM, :].rearrange(
                            "(a p) c -> p a c", p=128
                        ),
                        in_=zloc[:, mh * HC:(mh + 1) * HC, :],
                    )
                    if mh == MH - 1:
                        nc.gpsimd.collective_compute(
                            "AllGather", mybir.AluOpType.bypass,
                            replica_groups=rg,
                            ins=[z_in[:, :]], outs=[z_out[:, :]],
                        )

                # interleave m-blocks through the S-quarter-gated phase (fills
                # the arrival gaps), then finish m-block 0 first so its z rows
                # ship in the early z-AllGather while m-block 1's tail runs.
                STAIL = K - NCORES * (SB[-1] - SB[-2])
                for s in range(STAIL):
                    for mh in range(MH):
                        l1_mm(mh, s)
                for s in range(STAIL, K):
                    l1_mm(0, s)
                z_half(0)
                for s in range(STAIL, K):
                    l1_mm(1, s)
                z_half(1)
            astage_ctx.__exit__(None, None, None)

            # ---- PE warm-keeper: discardable matmuls spanning the z-AllGather
            # gap so HAM keeps the PE at full clock for layer 2. ----
            with tc.tile_pool(name="wpsum", bufs=1, space="PSUM") as wp:
                wps = wp.tile([128, 128], F32)
                for i in range(WARM_N):
                    nc.tensor.matmul(
                        wps[:, :],
                        w1bf[:, 0, :],
                        hT[:, 0:128],
                        start=True, stop=True,
                    )

            nc.sync.dma_start(
                out=zres[:, :, :],
                in_=z_out.rearrange("(k p) c -> p k c", p=128),
            )

            # ---- layer 2: logitsT += z_k^T @ adjT_k ----
            assert MH == 2
            with tc.tile_pool(name="lpsum", bufs=1, space="PSUM") as lp:
                if L2_COLTILE:
                    # one PSUM bank, m-block 0 -> partitions 0..39 (col-group
                    # 0/1), m-block 1 -> partitions 64..103 (col-group 2/3);
                    # the two matmuls per k stream concurrently. Slots 0..31
                    # hold k%MC<JH chunks, matching the first z-AllGather half.
                    pl = lp.tile([128, MW], F32)
                    for s in range(K):
                        k = korder[s]
                        nc.tensor.matmul(
                            pl[0:NCLASS, :],
                            zres[:, k, :],
                            adjres[:, s * M:s * M + MW],
                            start=(s == 0), stop=(s == K - 1),
                            tile_position=(0, 0), skip_group_check=True,
                        )
                        nc.tensor.matmul(
                            pl[64:64 + NCLASS, :],
                            zres[:, k, :],
                            adjres[:, s * M + MW:s * M + 2 * MW],
                            start=(s == 0), stop=(s == K - 1),
                            tile_position=(0, 64), skip_group_check=True,
                        )
                    nc.scalar.activation(
                        lTsb[:, 0:MW], pl[0:NCLASS, :],
                        mybir.ActivationFunctionType.Identity,
                        bias=b2sb[:, 0:1], scale=1.0,
                    )
                    nc.scalar.activation(
                        lThi[64:64 + NCLASS, :], pl[64:64 + NCLASS, :],
                        mybir.ActivationFunctionType.Identity,
                        bias=b2hi[64:64 + NCLASS, 0:1], scale=1.0,
                    )
                else:
                    pl = [lp.tile([NCLASS, MW], F32, name=f"pl{m}") for m in range(MH)]
                    for s in range(K):
                        k = korder[s]
                        for mh in range(MH):
                            nc.tensor.matmul(
                                pl[mh][:, :],
                                zres[:, k, :],
                                adjres[:, s * M + mh * MW:s * M + (mh + 1) * MW],
                                start=(s == 0), stop=(s == K - 1),
                            )
                    for mh in range(MH):
                        nc.scalar.activation(
                            lTsb[:, mh * MW:(mh + 1) * MW] if mh == 0
                            else lThi[64:64 + NCLASS, :],
                            pl[mh][:, :],
                            mybir.ActivationFunctionType.Identity,
                            bias=(b2sb[:, 0:1] if mh == 0
                                  else b2hi[64:64 + NCLASS, 0:1]),
                            scale=1.0,
                        )

            # ---- log_softmax over classes, batched per activation function so
            # the scalar engine loads each ACT table once ----
            with (
                tc.tile_pool(name="smp", bufs=1, space="PSUM") as smp,
                tc.tile_pool(name="sms", bufs=1) as sms,
            ):
                ptrs = [smp.tile([128, NCLASS], F32, name=f"ptr{m}") for m in range(MC)]
                mx = sms.tile([128, MC], F32)
                ssum = sms.tile([128, MC], F32)
                lse = sms.tile([128, MC], F32)
                bias2 = sms.tile([128, MC], F32)
                esc = sms.tile([128, NCLASS], F32)
                half = MC // 2
                for mc in range(MC):
                    if mc < half:
                        nc.tensor.transpose(
                            ptrs[mc][:, :], lTsb[:, mc * 128:(mc + 1) * 128],
                            ident[0:NCLASS, 0:NCLASS],
                        )
                    else:
                        nc.tensor.transpose(
                            ptrs[mc][:, :],
                            lThi[64:64 + NCLASS, (mc - half) * 128:(mc - half + 1) * 128],
                            ident2[64:64 + NCLASS, :],
                        )
                for mc in range(MC):
                    nc.vector.tensor_reduce(
                        mx[:, mc:mc + 1], ptrs[mc][:, :], axis=mybir.AxisListType.X,
                        op=mybir.AluOpType.max, negate=True,
                    )
                for mc in range(MC):
                    nc.scalar.activation(
                        esc[:, :], ptrs[mc][:, :], mybir.ActivationFunctionType.Exp,
                        bias=mx[:, mc:mc + 1], scale=1.0,
                        accum_out=ssum[:, mc:mc + 1],
                    )
                nc.scalar.activation(
                    lse[:, :], ssum[:, :], mybir.ActivationFunctionType.Ln,
                )
                nc.vector.tensor_sub(bias2[:, :], mx[:, :], lse[:, :])
                for mc in range(MC):
                    nc.vector.tensor_scalar_add(
                        osb[:, mc, :], ptrs[mc][:, :], bias2[:, mc:mc + 1],
                    )
            nc.sync.dma_start(
                out=out_ext.ap().rearrange("(a p) c -> p a c", p=128),
                in_=osb[:, :, :],
            )

    nc.compile()
    return nc


_NC_CACHE = {}


def _get_nc(n_total: int):
    if n_total not in _NC_CACHE:
        _NC_CACHE[n_total] = build(n_total)
    return _NC_CACHE[n_total]


def _permute_rows(a: np.ndarray, kk: int) -> np.ndarray:
    """Reorder rows so the device's "(p a) m" DMA layout reconstructs the
    natural "(a p) m" k-chunk layout with KK*4KB-contiguous descriptors."""
    n, m = a.shape
    nblk = n // (128 * kk)
    return np.ascontiguousarray(
        a.reshape(nblk, kk, 128, m).transpose(0, 2, 1, 3).reshape(n, m)
    )


def make_in_maps(x, adj, W1, b1, W2, b2):
    n_total = x.shape[0]
    m = n_total // NCORES
    K = n_total // 128
    MC = m // 128
    korder = _korder(K, MC)
    in_maps = []
    for i in range(NCORES):
        r0 = i * m
        ac = np.ascontiguousarray(adj[:, r0:r0 + m])
        # stream chunks in the layer-1 consumption order (korder), then the
        # within-chunk "(p a)" permutation for contiguous descriptors
        ac = ac.reshape(K, 128, m)[korder].reshape(n_total, m)
        in_maps.append({
            "xt": np.ascontiguousarray(x[r0:r0 + m].T),
            "adjc": _permute_rows(ac, KK),
            "w1": np.ascontiguousarray(W1),
            "b1": np.ascontiguousarray(b1.reshape(NHID, 1)),
            "w2": np.ascontiguousarray(W2),
            "b2": np.ascontiguousarray(b2.reshape(NCLASS, 1)),
        })
    return in_maps


def kernel(x, adj, W1, b1, W2, b2):
    x = np.asarray(x, dtype=np.float32)
    adj = np.asarray(adj, dtype=np.float32)
    W1 = np.asarray(W1, dtype=np.float32)
    b1 = np.asarray(b1, dtype=np.float32)
    W2 = np.asarray(W2, dtype=np.float32)
    b2 = np.asarray(b2, dtype=np.float32)
    nc = _get_nc(x.shape[0])
    in_maps = make_in_maps(x, adj, W1, b1, W2, b2)
    res = run_bass_kernel_spmd(nc, in_maps, list(range(NCORES)))
    return np.concatenate([res.results[i]["out"] for i in range(NCORES)], axis=0)
